# revision 5
# baseline (speedup 1.0000x reference)
"""Trainium2 Bass kernel for DGL HGNNConv-style hypergraph message passing.

Computation (see problem reference):
    Xp = X @ Wlin                                   # [N, 128] @ [128, 128]
    Xe = segment_sum(Xp[g1_src], g1_dst, 25000)     # node -> hyperedge
    Xe = Xe * degE * W
    Xv = segment_sum(Xe[g1_dst], g1_src, 100000)    # hyperedge -> node
    Xv = Xv * degV
tolerance is loose (2e-2) so the gather path runs in bf16; accumulation
stays fp32 in PSUM.

Distribution strategy (8 NeuronCores, node-range sharding):
  - Core m owns node rows [m*12500, (m+1)*12500) and all nnz entries whose
    src falls in that range (both stages use the same entry sharding).
  - Projection: each core computes Xp (bf16) for its own node shard.
  - Stage 1: per-core entries sorted by dst; rows of the local Xp gathered
    per entry (dma_gather, 256B bf16 rows), segment-summed into a
    full-range partial Xe via data-dependent one-hot matmuls (PSUM f32
    accumulation), scaled by degE*W -> bf16, then AllReduced across cores
    (4 chunks, overlapped with stage-1 compute).
  - Stage 2: per-core entries sorted by src; rows of the reduced Xe
    gathered per entry, segment-summed into the core's node tile,
    scaled by degV, written f32 to the core's output shard.

Gathers are issued in large multi-tile groups (up to GROUP chunks of 128
rows each) to amortize the ~1us fixed SWDGE descriptor-generation cost;
the SWDGE descriptor ring is enlarged (dynamic_dma_scratch_size) to allow
up to SUBG*128 descriptors per instruction, and gathers alternate between
two SWDGE queues.
"""

import numpy as np

import concourse.bass as bass
import concourse.bacc as bacc
import concourse.tile as tile
import concourse.mybir as mybir
from concourse.bass_utils import run_bass_kernel_spmd
from concourse.masks import make_identity

P = 128
N_CORES = 8

N_NODES = 100000
N_HEDGES = 25000
IN_CH = 128
OUT_CH = 128
N_AR_CHUNKS = 4    # AllReduce split for overlap with stage-1 compute
GROUP = 16         # chunk budget per work group (one-hot/gather batch)
SUBG = 4           # max chunks per dma_gather instruction
SCRATCH = 16384    # SWDGE descriptor-ring carveout bytes/partition
NQ = 1             # SWDGE queues; gathers alternate between them
USE_COLLECTIVE = True


def _cdiv(a, b):
    return (a + b - 1) // b


def _wrap_idx16(idx_flat: np.ndarray) -> np.ndarray:
    """Pack a flat index array into the [128, n/16] int16 SBUF layout used
    by dma_gather: flat index i -> partition i%16, column i//16, replicated
    across the eight 16-partition stripes."""
    n = idx_flat.shape[0]
    assert n % 16 == 0
    blk = idx_flat.astype(np.int16).reshape(n // 16, 16).T  # [16, cols]
    return np.tile(blk, (8, 1))  # [128, cols]


def _prep_stage(tile_key, gather_idx, local_id, n_tiles, n_cores):
    """Build per-core padded gather-index / segment-id arrays with a chunk
    schedule that is uniform across cores (SPMD requires one program).

    tile_key: per-core arrays of the tile id per entry (nondecreasing).
    Returns (chunks [n_tiles], idx_wrapped list, ids list).
    """
    counts = np.zeros((n_cores, n_tiles), dtype=np.int64)
    slices = []
    for c in range(n_cores):
        bounds = np.searchsorted(tile_key[c], np.arange(n_tiles + 1),
                                 side="left")
        counts[c] = bounds[1:] - bounds[:-1]
        slices.append(bounds)
    chunks = np.maximum(1, _cdiv(counts.max(axis=0), P)).astype(np.int64)
    total_chunks = int(chunks.sum())
    total = total_chunks * P
    co = np.concatenate([[0], np.cumsum(chunks)])

    idx_w, ids_w = [], []
    for c in range(n_cores):
        idx_flat = np.zeros(total, dtype=np.int16)
        ids_flat = np.full(total, -1.0, dtype=np.float32)
        bounds = slices[c]
        gi, li = gather_idx[c], local_id[c]
        for t in range(n_tiles):
            lo, hi = bounds[t], bounds[t + 1]
            cnt = hi - lo
            base = int(co[t]) * P
            idx_flat[base:base + cnt] = gi[lo:hi]
            ids_flat[base:base + cnt] = li[lo:hi]
        idx_w.append(_wrap_idx16(idx_flat))
        ids_w.append(np.ascontiguousarray(
            ids_flat.reshape(total_chunks, P).T))  # [128, total_chunks]
    return chunks, idx_w, ids_w


def _make_groups(chunks, t_lo, t_hi, budget=GROUP):
    """Greedy pack consecutive tiles [t_lo, t_hi) into groups with total
    chunk count <= budget (a single tile may exceed the budget and then
    forms its own group). Returns list of (t_start, t_end)."""
    groups = []
    t = t_lo
    while t < t_hi:
        e = t + 1
        tot = int(chunks[t])
        while e < t_hi and tot + int(chunks[e]) <= budget:
            tot += int(chunks[e])
            e += 1
        groups.append((t, e))
        t = e
    return groups


def _sub_splits(n, cap=SUBG):
    """Split n chunks into balanced pieces each <= cap."""
    k = _cdiv(n, cap)
    base = n // k
    rem = n % k
    return [base + (1 if i < rem else 0) for i in range(k)]


def _build_program(ns_pad, seg_pad, chunks1, chunks2, n_cores):
    """Emit the SPMD Bass program (identical for all cores)."""
    n_tiles_proj = ns_pad // P
    n_seg_tiles = seg_pad // P
    n_node_tiles = ns_pad // P
    tc1 = int(chunks1.sum())
    tc2 = int(chunks2.sum())
    co1 = np.concatenate([[0], np.cumsum(chunks1)]).astype(int)
    co2 = np.concatenate([[0], np.cumsum(chunks2)]).astype(int)

    nc = bacc.Bacc("TRN2", target_bir_lowering=False, debug=False,
                   num_devices=n_cores, dynamic_dma_scratch_size=SCRATCH,
                   num_swdge_queues=NQ)

    x_shard = nc.dram_tensor("x_shard", [ns_pad, IN_CH], mybir.dt.float32,
                             kind="ExternalInput")
    wlin = nc.dram_tensor("wlin", [IN_CH, OUT_CH], mybir.dt.float32,
                          kind="ExternalInput")
    dege_r = nc.dram_tensor("dege_r", [P, n_seg_tiles], mybir.dt.float32,
                            kind="ExternalInput")
    w_r = nc.dram_tensor("w_r", [P, n_seg_tiles], mybir.dt.float32,
                         kind="ExternalInput")
    degv_r = nc.dram_tensor("degv_r", [P, n_node_tiles], mybir.dt.float32,
                            kind="ExternalInput")
    colidx_in = nc.dram_tensor("colidx", [P, P], mybir.dt.float32,
                               kind="ExternalInput")
    idx1_in = nc.dram_tensor("idx1", [P, tc1 * 8], mybir.dt.int16,
                             kind="ExternalInput")
    ids1_in = nc.dram_tensor("ids1", [P, tc1], mybir.dt.float32,
                             kind="ExternalInput")
    idx2_in = nc.dram_tensor("idx2", [P, tc2 * 8], mybir.dt.int16,
                             kind="ExternalInput")
    ids2_in = nc.dram_tensor("ids2", [P, tc2], mybir.dt.float32,
                             kind="ExternalInput")
    out_shard = nc.dram_tensor("out_shard", [ns_pad, OUT_CH],
                               mybir.dt.float32, kind="ExternalOutput")

    # AllReduce chunk row ranges (in seg tiles)
    n_ar = min(N_AR_CHUNKS, n_seg_tiles)
    q_tiles = [n_seg_tiles // n_ar] * n_ar
    for i in range(n_seg_tiles % n_ar):
        q_tiles[i] += 1
    q_tile_lo = np.concatenate([[0], np.cumsum(q_tiles)]).astype(int)

    qsel = [0]  # alternating SWDGE queue for gathers

    with tile.TileContext(nc) as tc:
        with (
            tc.tile_pool(name="const", bufs=1) as cpool,
            tc.tile_pool(name="work", bufs=3) as work,
            tc.tile_pool(name="small", bufs=3) as small,
            tc.tile_pool(name="psum", bufs=2, space="PSUM") as psum,
            tc.tile_pool(name="psacc", bufs=4, space="PSUM") as psacc,
            tc.tile_pool(name="dram", bufs=1, space="DRAM") as dram,
        ):
            # ---- preloads ----
            idx1_sb = cpool.tile([P, tc1 * 8], mybir.dt.int16)
            nc.sync.dma_start(idx1_sb[:], idx1_in[:])
            ids1_f = cpool.tile([P, tc1], mybir.dt.float32)
            nc.sync.dma_start(ids1_f[:], ids1_in[:])
            idx2_sb = cpool.tile([P, tc2 * 8], mybir.dt.int16)
            nc.sync.dma_start(idx2_sb[:], idx2_in[:])
            ids2_f = cpool.tile([P, tc2], mybir.dt.float32)
            nc.sync.dma_start(ids2_f[:], ids2_in[:])
            colidx_f = cpool.tile([P, P], mybir.dt.float32)
            nc.sync.dma_start(colidx_f[:], colidx_in[:])
            wlin_f = cpool.tile([P, OUT_CH], mybir.dt.float32)
            nc.sync.dma_start(wlin_f[:], wlin[:])
            degv_sb = cpool.tile([P, n_node_tiles], mybir.dt.float32)
            nc.sync.dma_start(degv_sb[:], degv_r[:])
            dege_sb = cpool.tile([P, n_seg_tiles], mybir.dt.float32)
            nc.sync.dma_start(dege_sb[:], dege_r[:])
            w_sb = cpool.tile([P, n_seg_tiles], mybir.dt.float32)
            nc.sync.dma_start(w_sb[:], w_r[:])

            # bf16 casts of the comparison operands + weights
            ids1_sb = cpool.tile([P, tc1], mybir.dt.bfloat16)
            nc.vector.tensor_copy(ids1_sb[:], ids1_f[:])
            ids2_sb = cpool.tile([P, tc2], mybir.dt.bfloat16)
            nc.vector.tensor_copy(ids2_sb[:], ids2_f[:])
            colidx_sb = cpool.tile([P, P], mybir.dt.bfloat16)
            nc.vector.tensor_copy(colidx_sb[:], colidx_f[:])
            wlin_sb = cpool.tile([P, OUT_CH], mybir.dt.bfloat16)
            nc.vector.tensor_copy(wlin_sb[:], wlin_f[:])

            scale_e = cpool.tile([P, n_seg_tiles], mybir.dt.float32)
            nc.vector.tensor_tensor(out=scale_e[:], in0=dege_sb[:],
                                    in1=w_sb[:], op=mybir.AluOpType.mult)
            ident = cpool.tile([P, P], mybir.dt.float32)
            make_identity(nc, ident[:])
            colidx3 = colidx_sb[:].rearrange("p (o e) -> p o e", o=1)

            xp_local = dram.tile([ns_pad, OUT_CH], mybir.dt.bfloat16)
            xe_part = [
                dram.tile([q_tiles[q] * P, OUT_CH], mybir.dt.bfloat16,
                          name=f"xe_part{q}")
                for q in range(n_ar)
            ]
            xe_full = dram.tile([seg_pad, OUT_CH], mybir.dt.bfloat16)
            xe_red = [
                dram.tile([q_tiles[q] * P, OUT_CH], mybir.dt.bfloat16,
                          name=f"xe_red{q}", addr_space="Shared")
                for q in range(n_ar)
            ]

            # ---- projection: xp_local = bf16(x_shard @ wlin) ----
            for t in range(n_tiles_proj):
                rows = slice(t * P, (t + 1) * P)
                xt = small.tile([P, IN_CH], mybir.dt.float32, tag="xt")
                nc.sync.dma_start(xt[:], x_shard[rows, :])
                tp = psum.tile([P, P], mybir.dt.float32, space="PSUM",
                               tag="tp")
                nc.tensor.transpose(tp[:], xt[:], ident[:])
                xts = small.tile([P, P], mybir.dt.bfloat16, tag="xts")
                nc.vector.tensor_copy(xts[:], tp[:])
                xpp = psum.tile([P, OUT_CH], mybir.dt.float32, space="PSUM",
                                tag="xpp")
                nc.tensor.matmul(xpp[:], xts[:], wlin_sb[:], start=True,
                                 stop=True)
                xps = small.tile([P, OUT_CH], mybir.dt.bfloat16, tag="xps")
                nc.vector.tensor_copy(xps[:], xpp[:])
                nc.sync.dma_start(xp_local[rows, :], xps[:])

            # ---- generic grouped segment-sum stage ----
            def seg_group(t0, t1, chunks, co, idx_sb, ids_sb, src_ap,
                          scale_sb, ev_dtype, out_ap3):
                """Process tiles [t0, t1): one gather batch + one one-hot
                build + per-tile PSUM matmul accumulation; batched output
                write. out_ap3: [128, t1-t0, OUT_CH] DRAM view."""
                clo, chi = int(co[t0]), int(co[t1])
                ch = chi - clo
                n = ch * P
                g = work.tile([P, n], mybir.dt.bfloat16, tag="g")
                sub = 0
                for cw in _sub_splits(ch):
                    gs = g[:, sub * P:(sub + cw) * P].rearrange(
                        "p (c e) -> p c e", e=P)
                    nc.gpsimd.dma_gather(
                        gs, src_ap,
                        idx_sb[:, (clo + sub) * 8:(clo + sub + cw) * 8],
                        cw * P, cw * P, P, queue_num=qsel[0])
                    qsel[0] = (qsel[0] + 1) % NQ
                    sub += cw
                s = work.tile([P, n], mybir.dt.bfloat16, tag="s")
                s3 = s[:].rearrange("p (c e) -> p c e", e=P)
                nc.vector.tensor_tensor(
                    out=s3,
                    in0=ids_sb[:, clo:chi].to_broadcast([P, ch, P]),
                    in1=colidx3.to_broadcast([P, ch, P]),
                    op=mybir.AluOpType.is_equal,
                )
                ev = work.tile([P, (t1 - t0) * OUT_CH], ev_dtype, tag="ev")
                for t in range(t0, t1):
                    rel_lo = int(co[t]) - clo
                    rel_hi = int(co[t + 1]) - clo
                    acc = psacc.tile([P, OUT_CH], mybir.dt.float32,
                                     space="PSUM", tag="acc")
                    for c in range(rel_lo, rel_hi):
                        nc.tensor.matmul(
                            acc[:], s[:, c * P:(c + 1) * P],
                            g[:, c * P:(c + 1) * P],
                            start=(c == rel_lo), stop=(c == rel_hi - 1))
                    nc.vector.tensor_scalar_mul(
                        ev[:, (t - t0) * OUT_CH:(t - t0 + 1) * OUT_CH],
                        acc[:], scale_sb[:, t:t + 1])
                ev3 = ev[:].rearrange("p (g e) -> p g e", e=OUT_CH)
                nc.sync.dma_start(out_ap3, ev3)

            # ---- stage 1 (+ chunked AllReduce) ----
            for q in range(n_ar):
                for (t0, t1) in _make_groups(chunks1, q_tile_lo[q],
                                             q_tile_lo[q + 1]):
                    rel = t0 - q_tile_lo[q]
                    out3 = xe_part[q][rel * P:(rel + (t1 - t0)) * P, :] \
                        .rearrange("(g p) e -> p g e", p=P)
                    seg_group(t0, t1, chunks1, co1, idx1_sb, ids1_sb,
                              xp_local[:], scale_e, mybir.dt.bfloat16, out3)
                if USE_COLLECTIVE:
                    nc.gpsimd.collective_compute(
                        "AllReduce", mybir.AluOpType.add,
                        replica_groups=[list(range(n_cores))],
                        ins=[xe_part[q].opt()],
                        outs=[xe_red[q].opt()],
                    )
                    nc.sync.dma_start(
                        xe_full[q_tile_lo[q] * P:q_tile_lo[q + 1] * P, :],
                        xe_red[q][:])
                else:
                    nc.sync.dma_start(
                        xe_full[q_tile_lo[q] * P:q_tile_lo[q + 1] * P, :],
                        xe_part[q][:])

            # ---- stage 2 ----
            for (t0, t1) in _make_groups(chunks2, 0, n_node_tiles):
                out3 = out_shard[t0 * P:t1 * P, :] \
                    .rearrange("(g p) e -> p g e", p=P)
                seg_group(t0, t1, chunks2, co2, idx2_sb, ids2_sb,
                          xe_full[:], degv_sb, mybir.dt.float32, out3)

    nc.compile()
    return nc


def _host_prep(X, Wlin, degE, degV, W, g1_src, g1_dst, n_cores=N_CORES):
    ns = N_NODES // n_cores
    ns_pad = _cdiv(ns, P) * P
    n_seg_tiles = _cdiv(N_HEDGES, P)
    seg_pad = n_seg_tiles * P
    n_node_tiles = ns_pad // P

    core_of = g1_src // ns

    # stage 1: per core, sorted by dst
    o1 = np.lexsort((g1_dst, core_of))
    src1, dst1, c1 = g1_src[o1], g1_dst[o1], core_of[o1]
    cb1 = np.searchsorted(c1, np.arange(n_cores + 1))
    tile_key1, gidx1, lid1 = [], [], []
    for c in range(n_cores):
        lo, hi = cb1[c], cb1[c + 1]
        d = dst1[lo:hi]
        tile_key1.append(d // P)
        gidx1.append(src1[lo:hi] - c * ns)
        lid1.append((d % P).astype(np.float32))
    chunks1, idx1_w, ids1_w = _prep_stage(
        tile_key1, gidx1, lid1, n_seg_tiles, n_cores)

    # stage 2: per core, sorted by src
    o2 = np.argsort(g1_src, kind="stable")
    src2, dst2 = g1_src[o2], g1_dst[o2]
    cb2 = np.searchsorted(src2, np.arange(n_cores + 1) * ns)
    tile_key2, gidx2, lid2 = [], [], []
    for c in range(n_cores):
        lo, hi = cb2[c], cb2[c + 1]
        s_local = src2[lo:hi] - c * ns
        tile_key2.append(s_local // P)
        gidx2.append(dst2[lo:hi])
        lid2.append((s_local % P).astype(np.float32))
    chunks2, idx2_w, ids2_w = _prep_stage(
        tile_key2, gidx2, lid2, n_node_tiles, n_cores)

    # rearranged scale vectors
    def col_tiles(v, pad_rows):
        vp = np.zeros(pad_rows, dtype=np.float32)
        vp[:v.shape[0]] = v.reshape(-1)
        return np.ascontiguousarray(vp.reshape(pad_rows // P, P).T)

    dege_r = col_tiles(degE, seg_pad)
    w_r = col_tiles(W, seg_pad)
    colidx = np.broadcast_to(np.arange(P, dtype=np.float32), (P, P)).copy()

    in_maps = []
    for c in range(n_cores):
        xs = np.zeros((ns_pad, IN_CH), dtype=np.float32)
        xs[:ns] = X[c * ns:(c + 1) * ns]
        in_maps.append({
            "x_shard": xs,
            "wlin": np.ascontiguousarray(Wlin, dtype=np.float32),
            "dege_r": dege_r,
            "w_r": w_r,
            "degv_r": col_tiles(degV[c * ns:(c + 1) * ns], ns_pad),
            "colidx": colidx,
            "idx1": idx1_w[c],
            "ids1": ids1_w[c],
            "idx2": idx2_w[c],
            "ids2": ids2_w[c],
        })
    return in_maps, chunks1, chunks2, ns, ns_pad, seg_pad


def run_impl(inputs: dict, trace: bool = False):
    X = np.asarray(inputs["X"], dtype=np.float32)
    Wlin = np.asarray(inputs["Wlin"], dtype=np.float32)
    degE = np.asarray(inputs["degE"], dtype=np.float32)
    degV = np.asarray(inputs["degV"], dtype=np.float32)
    W = np.asarray(inputs["W"], dtype=np.float32)
    g1_src = np.asarray(inputs["g1_src"], dtype=np.int64)
    g1_dst = np.asarray(inputs["g1_dst"], dtype=np.int64)

    in_maps, chunks1, chunks2, ns, ns_pad, seg_pad = _host_prep(
        X, Wlin, degE, degV, W, g1_src, g1_dst)
    nc = _build_program(ns_pad, seg_pad, chunks1, chunks2, N_CORES)
    res = run_bass_kernel_spmd(nc, in_maps, core_ids=list(range(N_CORES)),
                               trace=trace)
    out = np.concatenate(
        [res.results[c]["out_shard"][:ns] for c in range(N_CORES)], axis=0)
    return out, res


def kernel(**inputs) -> np.ndarray:
    out, _ = run_impl(inputs, trace=False)
    return out


# revision 6
# speedup vs baseline: 2.8848x; 2.8848x over previous
"""Trainium2 Bass kernel for DGL HGNNConv-style hypergraph message passing.

Computation (see problem reference):
    Xp = X @ Wlin                                   # [N, 128] @ [128, 128]
    Xe = segment_sum(Xp[g1_src], g1_dst, 25000)     # node -> hyperedge
    Xe = Xe * degE * W
    Xv = segment_sum(Xe[g1_dst], g1_src, 100000)    # hyperedge -> node
    Xv = Xv * degV
tolerance is loose (2e-2) so the gather path runs in bf16; accumulation
stays fp32 in PSUM.

Distribution strategy (8 NeuronCores, node-range sharding):
  - Core m owns node rows [m*12500, (m+1)*12500) and all nnz entries whose
    src falls in that range (both stages use the same entry sharding).
  - Projection: each core computes Xp (bf16) for its own node shard.
  - Stage 1: per-core entries sorted by dst; rows of the local Xp gathered
    per entry (dma_gather, 256B bf16 rows), segment-summed into a
    full-range partial Xe via data-dependent one-hot matmuls (PSUM f32
    accumulation), scaled by degE*W -> bf16, then AllReduced across cores
    (4 chunks, overlapped with stage-1 compute).
  - Stage 2: per-core entries sorted by src; rows of the reduced Xe
    gathered per entry, segment-summed into the core's node tile,
    scaled by degV, written f32 to the core's output shard.

Gathers are issued in large multi-tile groups (up to GROUP chunks of 128
rows each) to amortize the ~1us fixed SWDGE descriptor-generation cost;
the SWDGE descriptor ring is enlarged (dynamic_dma_scratch_size) to allow
up to SUBG*128 descriptors per instruction, and gathers alternate between
two SWDGE queues.
"""

import numpy as np

import concourse.bass as bass
import concourse.bacc as bacc
import concourse.tile as tile
import concourse.mybir as mybir
from concourse.bass_utils import run_bass_kernel_spmd
from concourse.masks import make_identity

P = 128
N_CORES = 8

N_NODES = 100000
N_HEDGES = 25000
IN_CH = 128
OUT_CH = 128
N_AR_CHUNKS = 4    # AllReduce split for overlap with stage-1 compute
GROUP = 16         # chunk budget per work group (one-hot/gather batch)
SUBG = 4           # max chunks per dma_gather instruction
SCRATCH = 16384    # SWDGE descriptor-ring carveout bytes/partition
NQ = 4             # SWDGE queues; gathers round-robin across them
USE_COLLECTIVE = True


def _cdiv(a, b):
    return (a + b - 1) // b


def _wrap_idx16(idx_flat: np.ndarray) -> np.ndarray:
    """Pack a flat index array into the [128, n/16] int16 SBUF layout used
    by dma_gather: flat index i -> partition i%16, column i//16, replicated
    across the eight 16-partition stripes."""
    n = idx_flat.shape[0]
    assert n % 16 == 0
    blk = idx_flat.astype(np.int16).reshape(n // 16, 16).T  # [16, cols]
    return np.tile(blk, (8, 1))  # [128, cols]


def _prep_stage(tile_key, gather_idx, local_id, n_tiles, n_cores):
    """Build per-core padded gather-index / segment-id arrays with a chunk
    schedule that is uniform across cores (SPMD requires one program).

    tile_key: per-core arrays of the tile id per entry (nondecreasing).
    Returns (chunks [n_tiles], idx_wrapped list, ids list).
    """
    counts = np.zeros((n_cores, n_tiles), dtype=np.int64)
    slices = []
    for c in range(n_cores):
        bounds = np.searchsorted(tile_key[c], np.arange(n_tiles + 1),
                                 side="left")
        counts[c] = bounds[1:] - bounds[:-1]
        slices.append(bounds)
    chunks = np.maximum(1, _cdiv(counts.max(axis=0), P)).astype(np.int64)
    total_chunks = int(chunks.sum())
    total = total_chunks * P
    co = np.concatenate([[0], np.cumsum(chunks)])

    idx_w, ids_w = [], []
    for c in range(n_cores):
        idx_flat = np.zeros(total, dtype=np.int16)
        ids_flat = np.full(total, -1.0, dtype=np.float32)
        bounds = slices[c]
        gi, li = gather_idx[c], local_id[c]
        for t in range(n_tiles):
            lo, hi = bounds[t], bounds[t + 1]
            cnt = hi - lo
            base = int(co[t]) * P
            idx_flat[base:base + cnt] = gi[lo:hi]
            ids_flat[base:base + cnt] = li[lo:hi]
        idx_w.append(_wrap_idx16(idx_flat))
        ids_w.append(np.ascontiguousarray(
            ids_flat.reshape(total_chunks, P).T))  # [128, total_chunks]
    return chunks, idx_w, ids_w


def _make_groups(chunks, t_lo, t_hi, budget=GROUP):
    """Greedy pack consecutive tiles [t_lo, t_hi) into groups with total
    chunk count <= budget (a single tile may exceed the budget and then
    forms its own group). Returns list of (t_start, t_end)."""
    groups = []
    t = t_lo
    while t < t_hi:
        e = t + 1
        tot = int(chunks[t])
        while e < t_hi and tot + int(chunks[e]) <= budget:
            tot += int(chunks[e])
            e += 1
        groups.append((t, e))
        t = e
    return groups


def _sub_splits(n, cap=SUBG):
    """Split n chunks into balanced pieces each <= cap."""
    k = _cdiv(n, cap)
    base = n // k
    rem = n % k
    return [base + (1 if i < rem else 0) for i in range(k)]


def _build_program(ns_pad, seg_pad, chunks1, chunks2, n_cores):
    """Emit the SPMD Bass program (identical for all cores)."""
    n_tiles_proj = ns_pad // P
    n_seg_tiles = seg_pad // P
    n_node_tiles = ns_pad // P
    tc1 = int(chunks1.sum())
    tc2 = int(chunks2.sum())
    co1 = np.concatenate([[0], np.cumsum(chunks1)]).astype(int)
    co2 = np.concatenate([[0], np.cumsum(chunks2)]).astype(int)

    nc = bacc.Bacc("TRN2", target_bir_lowering=False, debug=False,
                   num_devices=n_cores, dynamic_dma_scratch_size=SCRATCH,
                   num_swdge_queues=NQ)

    x_shard = nc.dram_tensor("x_shard", [ns_pad, IN_CH], mybir.dt.float32,
                             kind="ExternalInput")
    wlin = nc.dram_tensor("wlin", [IN_CH, OUT_CH], mybir.dt.float32,
                          kind="ExternalInput")
    dege_r = nc.dram_tensor("dege_r", [P, n_seg_tiles], mybir.dt.float32,
                            kind="ExternalInput")
    w_r = nc.dram_tensor("w_r", [P, n_seg_tiles], mybir.dt.float32,
                         kind="ExternalInput")
    degv_r = nc.dram_tensor("degv_r", [P, n_node_tiles], mybir.dt.float32,
                            kind="ExternalInput")
    colidx_in = nc.dram_tensor("colidx", [P, P], mybir.dt.float32,
                               kind="ExternalInput")
    idx1_in = nc.dram_tensor("idx1", [P, tc1 * 8], mybir.dt.int16,
                             kind="ExternalInput")
    ids1_in = nc.dram_tensor("ids1", [P, tc1], mybir.dt.float32,
                             kind="ExternalInput")
    idx2_in = nc.dram_tensor("idx2", [P, tc2 * 8], mybir.dt.int16,
                             kind="ExternalInput")
    ids2_in = nc.dram_tensor("ids2", [P, tc2], mybir.dt.float32,
                             kind="ExternalInput")
    out_shard = nc.dram_tensor("out_shard", [ns_pad, OUT_CH],
                               mybir.dt.float32, kind="ExternalOutput")

    # AllReduce chunk row ranges (in seg tiles)
    n_ar = min(N_AR_CHUNKS, n_seg_tiles)
    q_tiles = [n_seg_tiles // n_ar] * n_ar
    for i in range(n_seg_tiles % n_ar):
        q_tiles[i] += 1
    q_tile_lo = np.concatenate([[0], np.cumsum(q_tiles)]).astype(int)

    qsel = [0]  # alternating SWDGE queue for gathers

    with tile.TileContext(nc) as tc:
        with (
            tc.tile_pool(name="const", bufs=1) as cpool,
            tc.tile_pool(name="work", bufs=3) as work,
            tc.tile_pool(name="small", bufs=3) as small,
            tc.tile_pool(name="psum", bufs=2, space="PSUM") as psum,
            tc.tile_pool(name="psacc", bufs=4, space="PSUM") as psacc,
            tc.tile_pool(name="dram", bufs=1, space="DRAM") as dram,
        ):
            # ---- preloads ----
            idx1_sb = cpool.tile([P, tc1 * 8], mybir.dt.int16)
            nc.sync.dma_start(idx1_sb[:], idx1_in[:])
            ids1_f = cpool.tile([P, tc1], mybir.dt.float32)
            nc.sync.dma_start(ids1_f[:], ids1_in[:])
            idx2_sb = cpool.tile([P, tc2 * 8], mybir.dt.int16)
            nc.sync.dma_start(idx2_sb[:], idx2_in[:])
            ids2_f = cpool.tile([P, tc2], mybir.dt.float32)
            nc.sync.dma_start(ids2_f[:], ids2_in[:])
            colidx_f = cpool.tile([P, P], mybir.dt.float32)
            nc.sync.dma_start(colidx_f[:], colidx_in[:])
            wlin_f = cpool.tile([P, OUT_CH], mybir.dt.float32)
            nc.sync.dma_start(wlin_f[:], wlin[:])
            degv_sb = cpool.tile([P, n_node_tiles], mybir.dt.float32)
            nc.sync.dma_start(degv_sb[:], degv_r[:])
            dege_sb = cpool.tile([P, n_seg_tiles], mybir.dt.float32)
            nc.sync.dma_start(dege_sb[:], dege_r[:])
            w_sb = cpool.tile([P, n_seg_tiles], mybir.dt.float32)
            nc.sync.dma_start(w_sb[:], w_r[:])

            # bf16 casts of the comparison operands + weights
            ids1_sb = cpool.tile([P, tc1], mybir.dt.bfloat16)
            nc.vector.tensor_copy(ids1_sb[:], ids1_f[:])
            ids2_sb = cpool.tile([P, tc2], mybir.dt.bfloat16)
            nc.vector.tensor_copy(ids2_sb[:], ids2_f[:])
            colidx_sb = cpool.tile([P, P], mybir.dt.bfloat16)
            nc.vector.tensor_copy(colidx_sb[:], colidx_f[:])
            wlin_sb = cpool.tile([P, OUT_CH], mybir.dt.bfloat16)
            nc.vector.tensor_copy(wlin_sb[:], wlin_f[:])

            scale_e = cpool.tile([P, n_seg_tiles], mybir.dt.float32)
            nc.vector.tensor_tensor(out=scale_e[:], in0=dege_sb[:],
                                    in1=w_sb[:], op=mybir.AluOpType.mult)
            ident = cpool.tile([P, P], mybir.dt.float32)
            make_identity(nc, ident[:])
            colidx3 = colidx_sb[:].rearrange("p (o e) -> p o e", o=1)

            xp_local = dram.tile([ns_pad, OUT_CH], mybir.dt.bfloat16)
            xe_part = [
                dram.tile([q_tiles[q] * P, OUT_CH], mybir.dt.bfloat16,
                          name=f"xe_part{q}")
                for q in range(n_ar)
            ]
            xe_full = dram.tile([seg_pad, OUT_CH], mybir.dt.bfloat16)
            xe_red = [
                dram.tile([q_tiles[q] * P, OUT_CH], mybir.dt.bfloat16,
                          name=f"xe_red{q}", addr_space="Shared")
                for q in range(n_ar)
            ]

            # ---- projection: xp_local = bf16(x_shard @ wlin) ----
            for t in range(n_tiles_proj):
                rows = slice(t * P, (t + 1) * P)
                xt = small.tile([P, IN_CH], mybir.dt.float32, tag="xt")
                nc.sync.dma_start(xt[:], x_shard[rows, :])
                tp = psum.tile([P, P], mybir.dt.float32, space="PSUM",
                               tag="tp")
                nc.tensor.transpose(tp[:], xt[:], ident[:])
                xts = small.tile([P, P], mybir.dt.bfloat16, tag="xts")
                nc.vector.tensor_copy(xts[:], tp[:])
                xpp = psum.tile([P, OUT_CH], mybir.dt.float32, space="PSUM",
                                tag="xpp")
                nc.tensor.matmul(xpp[:], xts[:], wlin_sb[:], start=True,
                                 stop=True)
                xps = small.tile([P, OUT_CH], mybir.dt.bfloat16, tag="xps")
                nc.vector.tensor_copy(xps[:], xpp[:])
                nc.sync.dma_start(xp_local[rows, :], xps[:])

            # ---- generic grouped segment-sum stage ----
            def seg_group(t0, t1, chunks, co, idx_sb, ids_sb, src_ap,
                          scale_sb, ev_dtype, out_ap3):
                """Process tiles [t0, t1): one gather batch + one one-hot
                build + per-tile PSUM matmul accumulation; batched output
                write. out_ap3: [128, t1-t0, OUT_CH] DRAM view."""
                clo, chi = int(co[t0]), int(co[t1])
                ch = chi - clo
                n = ch * P
                g = work.tile([P, n], mybir.dt.bfloat16, tag="g")
                sub = 0
                for cw in _sub_splits(ch):
                    gs = g[:, sub * P:(sub + cw) * P].rearrange(
                        "p (c e) -> p c e", e=P)
                    nc.gpsimd.dma_gather(
                        gs, src_ap,
                        idx_sb[:, (clo + sub) * 8:(clo + sub + cw) * 8],
                        cw * P, cw * P, P, queue_num=qsel[0])
                    qsel[0] = (qsel[0] + 1) % NQ
                    sub += cw
                s = work.tile([P, n], mybir.dt.bfloat16, tag="s")
                s3 = s[:].rearrange("p (c e) -> p c e", e=P)
                nc.vector.tensor_tensor(
                    out=s3,
                    in0=ids_sb[:, clo:chi].to_broadcast([P, ch, P]),
                    in1=colidx3.to_broadcast([P, ch, P]),
                    op=mybir.AluOpType.is_equal,
                )
                ev = work.tile([P, (t1 - t0) * OUT_CH], ev_dtype, tag="ev")
                for t in range(t0, t1):
                    rel_lo = int(co[t]) - clo
                    rel_hi = int(co[t + 1]) - clo
                    acc = psacc.tile([P, OUT_CH], mybir.dt.float32,
                                     space="PSUM", tag="acc")
                    for c in range(rel_lo, rel_hi):
                        nc.tensor.matmul(
                            acc[:], s[:, c * P:(c + 1) * P],
                            g[:, c * P:(c + 1) * P],
                            start=(c == rel_lo), stop=(c == rel_hi - 1))
                    nc.vector.tensor_scalar_mul(
                        ev[:, (t - t0) * OUT_CH:(t - t0 + 1) * OUT_CH],
                        acc[:], scale_sb[:, t:t + 1])
                ev3 = ev[:].rearrange("p (g e) -> p g e", e=OUT_CH)
                nc.sync.dma_start(out_ap3, ev3)

            # ---- stage 1 (+ chunked AllReduce) ----
            for q in range(n_ar):
                for (t0, t1) in _make_groups(chunks1, q_tile_lo[q],
                                             q_tile_lo[q + 1]):
                    rel = t0 - q_tile_lo[q]
                    out3 = xe_part[q][rel * P:(rel + (t1 - t0)) * P, :] \
                        .rearrange("(g p) e -> p g e", p=P)
                    seg_group(t0, t1, chunks1, co1, idx1_sb, ids1_sb,
                              xp_local[:], scale_e, mybir.dt.bfloat16, out3)
                if USE_COLLECTIVE:
                    nc.gpsimd.collective_compute(
                        "AllReduce", mybir.AluOpType.add,
                        replica_groups=[list(range(n_cores))],
                        ins=[xe_part[q].opt()],
                        outs=[xe_red[q].opt()],
                    )
                    nc.sync.dma_start(
                        xe_full[q_tile_lo[q] * P:q_tile_lo[q + 1] * P, :],
                        xe_red[q][:])
                else:
                    nc.sync.dma_start(
                        xe_full[q_tile_lo[q] * P:q_tile_lo[q + 1] * P, :],
                        xe_part[q][:])

            # ---- stage 2 ----
            for (t0, t1) in _make_groups(chunks2, 0, n_node_tiles):
                out3 = out_shard[t0 * P:t1 * P, :] \
                    .rearrange("(g p) e -> p g e", p=P)
                seg_group(t0, t1, chunks2, co2, idx2_sb, ids2_sb,
                          xe_full[:], degv_sb, mybir.dt.float32, out3)

    nc.compile()
    return nc


def _host_prep(X, Wlin, degE, degV, W, g1_src, g1_dst, n_cores=N_CORES):
    ns = N_NODES // n_cores
    ns_pad = _cdiv(ns, P) * P
    n_seg_tiles = _cdiv(N_HEDGES, P)
    seg_pad = n_seg_tiles * P
    n_node_tiles = ns_pad // P

    core_of = g1_src // ns

    # stage 1: per core, sorted by dst
    o1 = np.lexsort((g1_dst, core_of))
    src1, dst1, c1 = g1_src[o1], g1_dst[o1], core_of[o1]
    cb1 = np.searchsorted(c1, np.arange(n_cores + 1))
    tile_key1, gidx1, lid1 = [], [], []
    for c in range(n_cores):
        lo, hi = cb1[c], cb1[c + 1]
        d = dst1[lo:hi]
        tile_key1.append(d // P)
        gidx1.append(src1[lo:hi] - c * ns)
        lid1.append((d % P).astype(np.float32))
    chunks1, idx1_w, ids1_w = _prep_stage(
        tile_key1, gidx1, lid1, n_seg_tiles, n_cores)

    # stage 2: per core, sorted by src
    o2 = np.argsort(g1_src, kind="stable")
    src2, dst2 = g1_src[o2], g1_dst[o2]
    cb2 = np.searchsorted(src2, np.arange(n_cores + 1) * ns)
    tile_key2, gidx2, lid2 = [], [], []
    for c in range(n_cores):
        lo, hi = cb2[c], cb2[c + 1]
        s_local = src2[lo:hi] - c * ns
        tile_key2.append(s_local // P)
        gidx2.append(dst2[lo:hi])
        lid2.append((s_local % P).astype(np.float32))
    chunks2, idx2_w, ids2_w = _prep_stage(
        tile_key2, gidx2, lid2, n_node_tiles, n_cores)

    # rearranged scale vectors
    def col_tiles(v, pad_rows):
        vp = np.zeros(pad_rows, dtype=np.float32)
        vp[:v.shape[0]] = v.reshape(-1)
        return np.ascontiguousarray(vp.reshape(pad_rows // P, P).T)

    dege_r = col_tiles(degE, seg_pad)
    w_r = col_tiles(W, seg_pad)
    colidx = np.broadcast_to(np.arange(P, dtype=np.float32), (P, P)).copy()

    in_maps = []
    for c in range(n_cores):
        xs = np.zeros((ns_pad, IN_CH), dtype=np.float32)
        xs[:ns] = X[c * ns:(c + 1) * ns]
        in_maps.append({
            "x_shard": xs,
            "wlin": np.ascontiguousarray(Wlin, dtype=np.float32),
            "dege_r": dege_r,
            "w_r": w_r,
            "degv_r": col_tiles(degV[c * ns:(c + 1) * ns], ns_pad),
            "colidx": colidx,
            "idx1": idx1_w[c],
            "ids1": ids1_w[c],
            "idx2": idx2_w[c],
            "ids2": ids2_w[c],
        })
    return in_maps, chunks1, chunks2, ns, ns_pad, seg_pad


def run_impl(inputs: dict, trace: bool = False):
    X = np.asarray(inputs["X"], dtype=np.float32)
    Wlin = np.asarray(inputs["Wlin"], dtype=np.float32)
    degE = np.asarray(inputs["degE"], dtype=np.float32)
    degV = np.asarray(inputs["degV"], dtype=np.float32)
    W = np.asarray(inputs["W"], dtype=np.float32)
    g1_src = np.asarray(inputs["g1_src"], dtype=np.int64)
    g1_dst = np.asarray(inputs["g1_dst"], dtype=np.int64)

    in_maps, chunks1, chunks2, ns, ns_pad, seg_pad = _host_prep(
        X, Wlin, degE, degV, W, g1_src, g1_dst)
    nc = _build_program(ns_pad, seg_pad, chunks1, chunks2, N_CORES)
    res = run_bass_kernel_spmd(nc, in_maps, core_ids=list(range(N_CORES)),
                               trace=trace)
    out = np.concatenate(
        [res.results[c]["out_shard"][:ns] for c in range(N_CORES)], axis=0)
    return out, res


def kernel(**inputs) -> np.ndarray:
    out, _ = run_impl(inputs, trace=False)
    return out
